# revision 21
# baseline (speedup 1.0000x reference)
"""Contrastive volume loss (nn_ContrastiveVolumeLoss) on 8 Trainium2 cores.

Reference math:
  ind_k = floor(locations_k) @ [W, 1]
  G     = [emb_0.reshape(c,HW)[:, ind_0] | emb_1.reshape(c,HW)[:, ind_1]]
  sim   = G^T G                       (2n x 2n, G is channel-major (64, 8192))
  S_i   = sum_j exp(sim_ij / T) - e^(1/T)
  loss  = (sum_i log S_i - (2/T) sum_u sim[u, u+n]) / (2n)

Sharding: the host computes indices and slices the gathered point embeddings
(pure data staging). Each core owns 8 row-tiles of 128 rows (one per diagonal
work class, slot k's tile has diag region JD[k]); it computes the upper
trapezoid of the symmetric 8192x8192 sim matrix: slot k covers columns
[JD[k]*1024, 8192). Row sums of exp come from the activation accumulator or
DVE reduces; the lower triangle is recovered from column sums computed with
near-free tiny matmuls (exp-scratch as PE weights x ones, N=1) accumulated
in a pinned PSUM bank; the host merges rowsums + colsums + positive pairs.

Engine split (the scalar/Act engine is the PSUM-exp bottleneck; DVE is the
only other engine that can read PSUM):
  - Act: exact exp with fused row-sum accumulation on ~56% of columns.
  - DVE: Schraudolph approximate exp on the rest - one tensor_scalar per
    1024-col window computes int16 bits = sim*(T_inv*128/ln2) + 128*(127+s),
    which bitcast as bf16 is exp(sim/T) to ~1.8% rms (bias tuned via SIGMA).
  - Pool (gpsimd): bf16 add-trees combine 4 Schraudolph windows so DVE pays
    only one row-sum reduce per 4096 columns.
  - PE: bf16 sim matmuls + free colsum/pos tiny matmuls (cost model charges
    matmuls by output free size only).
"""

import numpy as np
import ml_dtypes

import concourse.bacc as bacc
import concourse.mybir as mybir
from concourse.tile import TileContext
from concourse.bass_utils import run_bass_kernel_spmd

N_CORES = 8
C = 64
HW = 256 * 256
N_PTS = 4096
TWO_N = 2 * N_PTS
T_INV = 10.0
W_IMG = 256

JD = [0, 7, 1, 6, 2, 5, 3, 4]      # diagonal region per slot
N_SLOTS = 8

SIGMA = -0.0579
A_CONST = float(T_INV * 128.0 / np.log(2.0))
B_CONST = float(128.0 * (127.0 + SIGMA))

_BF16 = ml_dtypes.bfloat16
_PROGRAM_CACHE = {}

# Per-slot window plan. 'A' windows -> Act exact exp (+accum rowsum);
# 'D' -> DVE Schraudolph window. Consecutive D windows of a slot form one
# group: Pool add-tree + a single DVE row-sum reduce. Sizes are columns
# relative to the slot start (global col JD[k]*1024). All windows <= 1536
# so every window streams through one double-buffered PSUM pool.
PLAN = {
    0: [("D", 1536), ("D", 1536), ("D", 1536), ("D", 1536),
        ("A", 1024), ("A", 1024)],
    1: [("A", 1024)],
    2: [("D", 1024), ("A", 1536), ("A", 1536), ("A", 1536), ("A", 1536)],
    3: [("A", 1024), ("A", 1024)],
    4: [("D", 1536), ("D", 1536), ("D", 1536), ("D", 1536)],
    5: [("A", 1536), ("A", 1536)],
    6: [("A", 1536), ("A", 1536), ("A", 1536), ("A", 512)],
    7: [("D", 1024), ("A", 1536), ("A", 1536)],
}

POS_OFF = 0      # 8 pos columns, then one column per (window, block) pair


def _slot_tiles(r):
    """Global 128-row tile indices owned by core r, in slot order."""
    return [r, 63 - r, 8 + r, 55 - r, 16 + r, 47 - r, 24 + r, 39 - r]


def _windows():
    """Flat window list: (slot, glo, width, kind, acc_col, group_id).
    Consecutive D windows of a slot share a group (one rowsum reduce)."""
    out = []
    group_size = {}
    for k in range(N_SLOTS):
        base = JD[k] * 1024
        off = 0
        acc_i = 0
        gid = None
        for kind, w in PLAN[k]:
            if kind == "A":
                out.append((k, base + off, w, "A", k * 8 + acc_i, None))
                acc_i += 1
                gid = None
            else:
                if gid is None:
                    gid = (k, acc_i)
                    group_size[gid] = 0
                    acc_i += 1
                out.append((k, base + off, w, "D", gid[0] * 8 + gid[1], gid))
                group_size[gid] += 1
            off += w
        assert JD[k] * 1024 + off == TWO_N
    return out, group_size


def _emission_order(wins):
    """Alternate D and A windows so Act and DVE both stream from the start;
    leftover A windows run at the end (pa is double-buffered, so
    consecutive same-engine windows still pipeline)."""
    d = [w for w in wins if w[3] == "D"]
    a = [w for w in wins if w[3] == "A"]
    order = []
    for i in range(max(len(d), len(a))):
        if i < len(d):
            order.append(d[i])
        if i < len(a):
            order.append(a[i])
    return order


def _cs_layout(order):
    """Assign each contributing (window, block) pair its own column of the
    pinned PSUM bank (no cross-instruction accumulation chains; the host
    sums the partials per global block). Returns {(wi, g): col}."""
    layout = {}
    col = 8  # cols 0..7 hold the pos dots
    for wi, (k, glo, w, kind, _, _) in enumerate(order):
        for b in range(w // 128):
            g = (glo // 128) + b
            if g // 8 == JD[k]:
                continue  # diagonal region: rowsum-only
            layout[(wi, g)] = col
            col += 1
    assert col <= 512
    return layout


def _build_program():
    nc = bacc.Bacc(
        "TRN2", target_bir_lowering=False, debug=False, num_devices=N_CORES
    )
    lhs_d = nc.dram_tensor("lhs", [C, 1024], mybir.dt.bfloat16,
                           kind="ExternalInput")
    rhs_d = nc.dram_tensor("rhs", [C, TWO_N], mybir.dt.bfloat16,
                           kind="ExternalInput")
    par_d = nc.dram_tensor("par", [C, 1024], mybir.dt.bfloat16,
                           kind="ExternalInput")
    ones_d = nc.dram_tensor("ones", [128, 1], mybir.dt.bfloat16,
                            kind="ExternalInput")
    wins, group_size = _windows()
    order = _emission_order(wins)
    layout = _cs_layout(order)
    n_cs_cols = 8 + len(layout)
    rs_d = nc.dram_tensor("rowsums", [128, 64], mybir.dt.float32,
                          kind="ExternalOutput")
    cs_d = nc.dram_tensor("colsums", [128, n_cs_cols], mybir.dt.float32,
                          kind="ExternalOutput")

    with TileContext(nc) as tc:
        with (
            tc.tile_pool(name="const", bufs=1) as cpool,
            tc.tile_pool(name="ascr", bufs=4) as apool,
            tc.tile_pool(name="dscr", bufs=8) as dpool,
            tc.tile_pool(name="addp", bufs=3) as addpool,
            tc.tile_pool(name="pin", bufs=1, space="PSUM") as pinpool,
            tc.tile_pool(name="pa", bufs=2, space="PSUM") as pa,
        ):
            # ---- constants / inputs ----
            lhs_t = cpool.tile([C, 1024], mybir.dt.bfloat16, tag="lhs")
            nc.scalar.dma_start(lhs_t[:], lhs_d[:])
            ones_t = cpool.tile([128, 1], mybir.dt.bfloat16, tag="ones")
            nc.scalar.dma_start(ones_t[:], ones_d[:])
            par_t = cpool.tile([C, 1024], mybir.dt.bfloat16, tag="par")
            nc.scalar.dma_start(par_t[:], par_d[:])
            rhs_t = cpool.tile([C, TWO_N], mybir.dt.bfloat16, tag="rhs")
            for j in range(4):
                nc.sync.dma_start(rhs_t[:, j * 2048:(j + 1) * 2048],
                                  rhs_d[:, j * 2048:(j + 1) * 2048])

            # exp act-table warm-up during the input DMA window
            warm_t = cpool.tile([1, 1], mybir.dt.float32, tag="warm")
            nc.gpsimd.memset(warm_t[:], 0.0)
            nc.scalar.activation(warm_t[:], warm_t[:],
                                 mybir.ActivationFunctionType.Exp, scale=1.0)

            # rowsum accumulator columns (<=4 per slot) and pinned colsum bank
            acc = cpool.tile([128, 64], mybir.dt.float32, tag="acc")
            nc.gpsimd.memset(acc[:], 0.0)
            cs_acc = pinpool.tile([128, 512], mybir.dt.float32, tag="cs")

            # PE p-state warm-up: keep the tensor engine continuously busy
            # from t~0 so real fills run at full clock once inputs land.
            wrow = cpool.tile([1, 512], mybir.dt.bfloat16, tag="wrow")
            nc.gpsimd.memset(wrow[:], 1.0)
            wps = pa.tile([128, 1536], mybir.dt.float32, tag="pa")
            for _ in range(7):
                nc.tensor.matmul(wps[:1, :512], wrow[:, :1], wrow[:],
                                 start=True, stop=True)

            # ---- positive pairs: prod = lhs .* par, tiny-matmul col dots ----
            prod_t = cpool.tile([C, 1024], mybir.dt.bfloat16, tag="prod")
            nc.vector.tensor_tensor(prod_t[:], lhs_t[:], par_t[:],
                                    mybir.AluOpType.mult)
            for b in range(8):
                nc.tensor.matmul(cs_acc[:, POS_OFF + b:POS_OFF + b + 1],
                                 prod_t[:, b * 128:(b + 1) * 128],
                                 ones_t[:C, :], start=True, stop=True)

            # ---- streamed windows ----
            # Tiny colsum matmuls and group reduces are deferred by LAG
            # windows: PE runs in program order, so emitting them inline
            # would stall the next tile's fill behind the exp that the
            # scratch depends on.
            group_scr = {}   # gid -> list of (width, int16 scratch tile)
            deferred = []
            LAG = 3
            for wi, (k, glo, w, kind, acol, gid) in enumerate(order):
                lhsT = lhs_t[:, k * 128:(k + 1) * 128]
                ptile = pa.tile([128, 1536], mybir.dt.float32, tag="pa")
                nmm = (w + 511) // 512
                for h in range(nmm):
                    cw = min(512, w - h * 512)
                    nc.tensor.matmul(ptile[:, h * 512:h * 512 + cw], lhsT,
                                     rhs_t[:, glo + h * 512:glo + h * 512 + cw],
                                     start=True, stop=True)

                if kind == "A":
                    scr = apool.tile([128, w], mybir.dt.bfloat16,
                                     tag=f"as{w}")
                    nc.scalar.activation(
                        scr[:], ptile[:, :w],
                        mybir.ActivationFunctionType.Exp, scale=T_INV,
                        accum_out=acc[:, acol:acol + 1])
                    scr_b = scr
                else:
                    scr = dpool.tile([128, w], mybir.dt.int16, tag=f"ds{w}")
                    nc.vector.tensor_scalar(
                        scr[:], ptile[:, :w], A_CONST, B_CONST,
                        mybir.AluOpType.mult, mybir.AluOpType.add)
                    group_scr.setdefault(gid, []).append((w, scr))
                    scr_b = None

                # colsum tiny matmuls (free on PE): scratch block as weights
                sb = scr[:].bitcast(mybir.dt.bfloat16) if kind == "D" else scr_b[:]

                def _tiny(sb=sb, k=k, glo=glo, w=w, wi=wi):
                    for b in range(w // 128):
                        g = (glo // 128) + b
                        if g // 8 == JD[k]:
                            continue
                        col = layout[(wi, g)]
                        nc.tensor.matmul(
                            cs_acc[:, col:col + 1],
                            sb[:, b * 128:(b + 1) * 128], ones_t[:],
                            start=True, stop=True)
                deferred.append(_tiny)

                # close out a finished D group: Pool add tree (Pool is
                # otherwise idle), deferred DVE reduce of the tree total.
                if gid is not None and len(group_scr.get(gid, ())) == group_size[gid]:
                    mem = group_scr.pop(gid)
                    if len(mem) == 1:
                        red_in = mem[0][1][:].bitcast(mybir.dt.bfloat16)
                    else:
                        assert len(mem) == 4
                        s0, s1, s2, s3 = [t[:].bitcast(mybir.dt.bfloat16)
                                          for _, t in mem]
                        gw = mem[0][0]
                        s01 = addpool.tile([128, gw], mybir.dt.bfloat16,
                                           tag="s01")
                        s23 = addpool.tile([128, gw], mybir.dt.bfloat16,
                                           tag="s23")
                        stot = addpool.tile([128, gw], mybir.dt.bfloat16,
                                            tag="stot")
                        nc.gpsimd.tensor_tensor(s01[:], s0, s1,
                                                mybir.AluOpType.add)
                        nc.gpsimd.tensor_tensor(s23[:], s2, s3,
                                                mybir.AluOpType.add)
                        nc.gpsimd.tensor_tensor(stot[:], s01[:], s23[:],
                                                mybir.AluOpType.add)
                        red_in = stot[:]

                    def _red(red_in=red_in, acol=acol):
                        nc.vector.tensor_reduce(acc[:, acol:acol + 1],
                                                red_in,
                                                axis=mybir.AxisListType.X,
                                                op=mybir.AluOpType.add)
                    deferred.append(_red)

                while len(deferred) > LAG:
                    deferred.pop(0)()

            for fn in deferred:
                fn()

            # ---- outputs ----
            cs_sb = cpool.tile([128, n_cs_cols], mybir.dt.float32,
                               tag="cs_sb")
            nc.vector.tensor_copy(cs_sb[:], cs_acc[:, :n_cs_cols])
            nc.sync.dma_start(rs_d[:], acc[:])
            nc.sync.dma_start(cs_d[:], cs_sb[:])

    nc.compile()
    nc._cs_layout_host = layout
    return nc


def kernel(emb_0, emb_1, locations_0, locations_1):
    emb_0 = np.asarray(emb_0)
    emb_1 = np.asarray(emb_1)
    locations_0 = np.asarray(locations_0)
    locations_1 = np.asarray(locations_1)

    strides = np.array([W_IMG, 1], dtype=np.float32)
    ind0 = (np.floor(locations_0[0]) @ strides).astype(np.int32)
    ind1 = (np.floor(locations_1[0]) @ strides).astype(np.int32)

    g0 = emb_0.reshape(C, HW)[:, ind0]
    g1 = emb_1.reshape(C, HW)[:, ind1]
    G = np.concatenate([g0, g1], axis=1).astype(_BF16)   # (64, 8192)
    P = np.concatenate([g1, g0], axis=1).astype(_BF16)   # partner columns

    if "nc" not in _PROGRAM_CACHE:
        _PROGRAM_CACHE["nc"] = _build_program()
    nc = _PROGRAM_CACHE["nc"]

    ones = np.ones((128, 1), dtype=_BF16)
    in_maps = []
    row_of = np.empty((N_CORES, 1024), dtype=np.int64)
    for r in range(N_CORES):
        tiles = _slot_tiles(r)
        rows = np.concatenate(
            [np.arange(mt * 128, (mt + 1) * 128) for mt in tiles])
        row_of[r] = rows
        in_maps.append({
            "lhs": np.ascontiguousarray(G[:, rows]),
            "rhs": G,
            "par": np.ascontiguousarray(P[:, rows]),
            "ones": ones,
        })

    res = run_bass_kernel_spmd(nc, in_maps, core_ids=list(range(N_CORES)))

    layout = _cs_layout(_emission_order(_windows()[0]))
    rowsum = np.zeros(TWO_N, dtype=np.float64)
    pos_total = 0.0
    for r in range(N_CORES):
        rows = row_of[r]
        rs = res.results[r]["rowsums"].astype(np.float64)   # (128, 64)
        for k in range(N_SLOTS):
            srow = rs[:, k * 8:(k + 1) * 8].sum(axis=1)     # (128,)
            rowsum[rows[k * 128:(k + 1) * 128]] += srow
        cs = res.results[r]["colsums"].astype(np.float64)
        for (_, g), col in layout.items():
            rowsum[g * 128:(g + 1) * 128] += cs[:, col]
        pos_total += float(cs[:, POS_OFF:POS_OFF + 8].sum())

    sums = rowsum - float(np.exp(np.float32(T_INV), dtype=np.float32))
    loss = (np.sum(np.log(sums)) - T_INV * pos_total) / TWO_N
    return np.float32(loss)


# revision 24
# speedup vs baseline: 1.0440x; 1.0440x over previous
"""Contrastive volume loss (nn_ContrastiveVolumeLoss) on 8 Trainium2 cores.

Reference math:
  ind_k = floor(locations_k) @ [W, 1]
  G     = [emb_0.reshape(c,HW)[:, ind_0] | emb_1.reshape(c,HW)[:, ind_1]]
  sim   = G^T G                       (2n x 2n, G is channel-major (64, 8192))
  S_i   = sum_j exp(sim_ij / T) - e^(1/T)
  loss  = (sum_i log S_i - (2/T) sum_u sim[u, u+n]) / (2n)

Sharding: the host computes indices and slices the gathered point embeddings
(pure data staging). Each core owns 8 row-tiles of 128 rows (one per diagonal
work class, slot k's tile has diag region JD[k]); it computes the upper
trapezoid of the symmetric 8192x8192 sim matrix: slot k covers columns
[JD[k]*1024, 8192). Row sums of exp come from the activation accumulator or
DVE reduces; the lower triangle is recovered from column sums computed with
near-free tiny matmuls (exp-scratch as PE weights x ones, N=1) accumulated
in a pinned PSUM bank; the host merges rowsums + colsums + positive pairs.

Engine split (the scalar/Act engine is the PSUM-exp bottleneck; DVE is the
only other engine that can read PSUM):
  - Act: exact exp with fused row-sum accumulation on ~56% of columns.
  - DVE: Schraudolph approximate exp on the rest - one tensor_scalar per
    1024-col window computes int16 bits = sim*(T_inv*128/ln2) + 128*(127+s),
    which bitcast as bf16 is exp(sim/T) to ~1.8% rms (bias tuned via SIGMA).
  - Pool (gpsimd): bf16 add-trees combine 4 Schraudolph windows so DVE pays
    only one row-sum reduce per 4096 columns.
  - PE: bf16 sim matmuls + free colsum/pos tiny matmuls (cost model charges
    matmuls by output free size only).
"""

import numpy as np
import ml_dtypes

import concourse.bacc as bacc
import concourse.mybir as mybir
from concourse.tile import TileContext
from concourse.bass_utils import run_bass_kernel_spmd

N_CORES = 8
C = 64
HW = 256 * 256
N_PTS = 4096
TWO_N = 2 * N_PTS
T_INV = 10.0
W_IMG = 256

JD = [0, 7, 1, 6, 2, 5, 3, 4]      # diagonal region per slot
N_SLOTS = 8

SIGMA = -0.0579
A_CONST = float(T_INV * 128.0 / np.log(2.0))
B_CONST = float(128.0 * (127.0 + SIGMA))

_BF16 = ml_dtypes.bfloat16
_PROGRAM_CACHE = {}

# Per-slot window plan. 'A' windows -> Act exact exp (+accum rowsum);
# 'D' -> DVE Schraudolph window. Consecutive D windows of a slot form one
# group: Pool add-tree + a single DVE row-sum reduce. Sizes are columns
# relative to the slot start (global col JD[k]*1024). All windows <= 1536
# so every window streams through one double-buffered PSUM pool.
PLAN = {
    0: [("D", 1024), ("D", 1024), ("D", 1024), ("D", 1024),
        ("A", 1024), ("A", 1024), ("A", 1024), ("A", 1024)],
    1: [("A", 1024)],
    2: [("D", 1024), ("D", 1024), ("A", 1024), ("A", 1024), ("A", 1024),
        ("A", 1024), ("A", 1024)],
    3: [("A", 1024), ("A", 1024)],
    4: [("D", 1024), ("D", 1024), ("D", 1024), ("D", 1024),
        ("A", 1024), ("A", 1024)],
    5: [("A", 1024), ("A", 1024), ("A", 1024)],
    6: [("D", 1024), ("D", 1024), ("A", 1024), ("A", 1024), ("A", 1024)],
    7: [("D", 1024), ("D", 1024), ("A", 1024), ("A", 1024)],
}

POS_OFF = 0      # 8 pos columns, then one column per (window, block) pair


def _slot_tiles(r):
    """Global 128-row tile indices owned by core r, in slot order."""
    return [r, 63 - r, 8 + r, 55 - r, 16 + r, 47 - r, 24 + r, 39 - r]


def _windows():
    """Flat window list: (slot, glo, width, kind, acc_col, group_id).
    Consecutive D windows of a slot share a group (one rowsum reduce)."""
    out = []
    group_size = {}
    for k in range(N_SLOTS):
        base = JD[k] * 1024
        off = 0
        acc_i = 0
        gid = None
        for kind, w in PLAN[k]:
            if kind == "A":
                out.append((k, base + off, w, "A", k * 8 + acc_i, None))
                acc_i += 1
                gid = None
            else:
                if gid is None:
                    gid = (k, acc_i)
                    group_size[gid] = 0
                    acc_i += 1
                out.append((k, base + off, w, "D", gid[0] * 8 + gid[1], gid))
                group_size[gid] += 1
            off += w
        assert JD[k] * 1024 + off == TWO_N
    return out, group_size


def _emission_order(wins):
    """Alternate D and A windows so Act and DVE both stream from the start;
    leftover A windows run at the end (pa is double-buffered, so
    consecutive same-engine windows still pipeline)."""
    d = [w for w in wins if w[3] == "D"]
    a = [w for w in wins if w[3] == "A"]
    order = []
    for i in range(max(len(d), len(a))):
        if i < len(d):
            order.append(d[i])
        if i < len(a):
            order.append(a[i])
    return order


def _cs_layout(order):
    """Assign each contributing (window, block) pair its own column of the
    pinned PSUM bank (no cross-instruction accumulation chains; the host
    sums the partials per global block). Returns {(wi, g): col}."""
    layout = {}
    col = 8  # cols 0..7 hold the pos dots
    for wi, (k, glo, w, kind, _, _) in enumerate(order):
        for b in range(w // 128):
            g = (glo // 128) + b
            if g // 8 == JD[k]:
                continue  # diagonal region: rowsum-only
            layout[(wi, g)] = col
            col += 1
    assert col <= 512
    return layout


def _build_program():
    nc = bacc.Bacc(
        "TRN2", target_bir_lowering=False, debug=False, num_devices=N_CORES
    )
    lhs_d = nc.dram_tensor("lhs", [C, 1024], mybir.dt.bfloat16,
                           kind="ExternalInput")
    rhs_d = nc.dram_tensor("rhs", [C, TWO_N], mybir.dt.bfloat16,
                           kind="ExternalInput")
    par_d = nc.dram_tensor("par", [C, 1024], mybir.dt.bfloat16,
                           kind="ExternalInput")
    ones_d = nc.dram_tensor("ones", [128, 1], mybir.dt.bfloat16,
                            kind="ExternalInput")
    wins, group_size = _windows()
    order = _emission_order(wins)
    layout = _cs_layout(order)
    n_cs_cols = 8 + len(layout)
    rs_d = nc.dram_tensor("rowsums", [128, 64], mybir.dt.float32,
                          kind="ExternalOutput")
    cs_d = nc.dram_tensor("colsums", [128, n_cs_cols], mybir.dt.float32,
                          kind="ExternalOutput")

    with TileContext(nc) as tc:
        with (
            tc.tile_pool(name="const", bufs=1) as cpool,
            tc.tile_pool(name="ascr", bufs=4) as apool,
            tc.tile_pool(name="dscr", bufs=8) as dpool,
            tc.tile_pool(name="addp", bufs=3) as addpool,
            tc.tile_pool(name="pin", bufs=1, space="PSUM") as pinpool,
            tc.tile_pool(name="pa", bufs=3, space="PSUM") as pa,
        ):
            # ---- constants / inputs ----
            lhs_t = cpool.tile([C, 1024], mybir.dt.bfloat16, tag="lhs")
            nc.scalar.dma_start(lhs_t[:], lhs_d[:])
            ones_t = cpool.tile([128, 1], mybir.dt.bfloat16, tag="ones")
            nc.scalar.dma_start(ones_t[:], ones_d[:])
            par_t = cpool.tile([C, 1024], mybir.dt.bfloat16, tag="par")
            nc.scalar.dma_start(par_t[:], par_d[:])
            rhs_t = cpool.tile([C, TWO_N], mybir.dt.bfloat16, tag="rhs")
            for j in range(4):
                nc.sync.dma_start(rhs_t[:, j * 2048:(j + 1) * 2048],
                                  rhs_d[:, j * 2048:(j + 1) * 2048])

            # exp act-table warm-up during the input DMA window
            warm_t = cpool.tile([1, 1], mybir.dt.float32, tag="warm")
            nc.gpsimd.memset(warm_t[:], 0.0)
            nc.scalar.activation(warm_t[:], warm_t[:],
                                 mybir.ActivationFunctionType.Exp, scale=1.0)

            # rowsum accumulator columns (<=4 per slot) and pinned colsum bank
            acc = cpool.tile([128, 64], mybir.dt.float32, tag="acc")
            nc.gpsimd.memset(acc[:], 0.0)
            cs_acc = pinpool.tile([128, 512], mybir.dt.float32, tag="cs")

            # PE p-state warm-up: keep the tensor engine continuously busy
            # from t~0 so real fills run at full clock once inputs land.
            wrow = cpool.tile([1, 512], mybir.dt.bfloat16, tag="wrow")
            nc.gpsimd.memset(wrow[:], 1.0)
            wps = pa.tile([128, 1024], mybir.dt.float32, tag="pa")
            for _ in range(7):
                nc.tensor.matmul(wps[:1, :512], wrow[:, :1], wrow[:],
                                 start=True, stop=True)

            # ---- positive pairs: prod = lhs .* par, tiny-matmul col dots ----
            prod_t = cpool.tile([C, 1024], mybir.dt.bfloat16, tag="prod")
            nc.gpsimd.tensor_tensor(prod_t[:], lhs_t[:], par_t[:],
                                    mybir.AluOpType.mult)
            for b in range(8):
                nc.tensor.matmul(cs_acc[:, POS_OFF + b:POS_OFF + b + 1],
                                 prod_t[:, b * 128:(b + 1) * 128],
                                 ones_t[:C, :], start=True, stop=True)

            # ---- streamed windows ----
            # Tiny colsum matmuls and group reduces are deferred by LAG
            # windows: PE runs in program order, so emitting them inline
            # would stall the next tile's fill behind the exp that the
            # scratch depends on.
            group_scr = {}   # gid -> list of (width, int16 scratch tile)
            deferred = []
            LAG = 3
            for wi, (k, glo, w, kind, acol, gid) in enumerate(order):
                lhsT = lhs_t[:, k * 128:(k + 1) * 128]
                ptile = pa.tile([128, 1024], mybir.dt.float32, tag="pa")
                nmm = (w + 511) // 512
                for h in range(nmm):
                    cw = min(512, w - h * 512)
                    nc.tensor.matmul(ptile[:, h * 512:h * 512 + cw], lhsT,
                                     rhs_t[:, glo + h * 512:glo + h * 512 + cw],
                                     start=True, stop=True)

                if kind == "A":
                    scr = apool.tile([128, w], mybir.dt.bfloat16,
                                     tag=f"as{w}")
                    nc.scalar.activation(
                        scr[:], ptile[:, :w],
                        mybir.ActivationFunctionType.Exp, scale=T_INV,
                        accum_out=acc[:, acol:acol + 1])
                    scr_b = scr
                else:
                    scr = dpool.tile([128, w], mybir.dt.int16, tag=f"ds{w}")
                    nc.vector.tensor_scalar(
                        scr[:], ptile[:, :w], A_CONST, B_CONST,
                        mybir.AluOpType.mult, mybir.AluOpType.add)
                    group_scr.setdefault(gid, []).append((w, scr))
                    scr_b = None

                # colsum tiny matmuls (free on PE): scratch block as weights
                sb = scr[:].bitcast(mybir.dt.bfloat16) if kind == "D" else scr_b[:]

                def _tiny(sb=sb, k=k, glo=glo, w=w, wi=wi):
                    for b in range(w // 128):
                        g = (glo // 128) + b
                        if g // 8 == JD[k]:
                            continue
                        col = layout[(wi, g)]
                        nc.tensor.matmul(
                            cs_acc[:, col:col + 1],
                            sb[:, b * 128:(b + 1) * 128], ones_t[:],
                            start=True, stop=True)
                deferred.append(_tiny)

                # close out a finished D group: Pool add tree (Pool is
                # otherwise idle), deferred DVE reduce of the tree total.
                if gid is not None and len(group_scr.get(gid, ())) == group_size[gid]:
                    mem = group_scr.pop(gid)
                    if len(mem) == 1:
                        red_in = mem[0][1][:].bitcast(mybir.dt.bfloat16)
                    elif len(mem) == 2:
                        gw = mem[0][0]
                        stot = addpool.tile([128, gw], mybir.dt.bfloat16,
                                            tag="stot")
                        nc.gpsimd.tensor_tensor(
                            stot[:], mem[0][1][:].bitcast(mybir.dt.bfloat16),
                            mem[1][1][:].bitcast(mybir.dt.bfloat16),
                            mybir.AluOpType.add)
                        red_in = stot[:]
                    else:
                        assert len(mem) == 4
                        s0, s1, s2, s3 = [t[:].bitcast(mybir.dt.bfloat16)
                                          for _, t in mem]
                        gw = mem[0][0]
                        s01 = addpool.tile([128, gw], mybir.dt.bfloat16,
                                           tag="s01")
                        s23 = addpool.tile([128, gw], mybir.dt.bfloat16,
                                           tag="s23")
                        stot = addpool.tile([128, gw], mybir.dt.bfloat16,
                                            tag="stot")
                        nc.gpsimd.tensor_tensor(s01[:], s0, s1,
                                                mybir.AluOpType.add)
                        nc.gpsimd.tensor_tensor(s23[:], s2, s3,
                                                mybir.AluOpType.add)
                        nc.gpsimd.tensor_tensor(stot[:], s01[:], s23[:],
                                                mybir.AluOpType.add)
                        red_in = stot[:]

                    def _red(red_in=red_in, acol=acol):
                        nc.vector.tensor_reduce(acc[:, acol:acol + 1],
                                                red_in,
                                                axis=mybir.AxisListType.X,
                                                op=mybir.AluOpType.add)
                    deferred.append(_red)

                while len(deferred) > LAG:
                    deferred.pop(0)()

            for fn in deferred:
                fn()

            # ---- outputs ----
            cs_sb = cpool.tile([128, n_cs_cols], mybir.dt.float32,
                               tag="cs_sb")
            nc.vector.tensor_copy(cs_sb[:], cs_acc[:, :n_cs_cols])
            nc.sync.dma_start(rs_d[:], acc[:])
            nc.sync.dma_start(cs_d[:], cs_sb[:])

    nc.compile()
    nc._cs_layout_host = layout
    return nc


def kernel(emb_0, emb_1, locations_0, locations_1):
    emb_0 = np.asarray(emb_0)
    emb_1 = np.asarray(emb_1)
    locations_0 = np.asarray(locations_0)
    locations_1 = np.asarray(locations_1)

    strides = np.array([W_IMG, 1], dtype=np.float32)
    ind0 = (np.floor(locations_0[0]) @ strides).astype(np.int32)
    ind1 = (np.floor(locations_1[0]) @ strides).astype(np.int32)

    g0 = emb_0.reshape(C, HW)[:, ind0]
    g1 = emb_1.reshape(C, HW)[:, ind1]
    G = np.concatenate([g0, g1], axis=1).astype(_BF16)   # (64, 8192)
    P = np.concatenate([g1, g0], axis=1).astype(_BF16)   # partner columns

    if "nc" not in _PROGRAM_CACHE:
        _PROGRAM_CACHE["nc"] = _build_program()
    nc = _PROGRAM_CACHE["nc"]

    ones = np.ones((128, 1), dtype=_BF16)
    in_maps = []
    row_of = np.empty((N_CORES, 1024), dtype=np.int64)
    for r in range(N_CORES):
        tiles = _slot_tiles(r)
        rows = np.concatenate(
            [np.arange(mt * 128, (mt + 1) * 128) for mt in tiles])
        row_of[r] = rows
        in_maps.append({
            "lhs": np.ascontiguousarray(G[:, rows]),
            "rhs": G,
            "par": np.ascontiguousarray(P[:, rows]),
            "ones": ones,
        })

    res = run_bass_kernel_spmd(nc, in_maps, core_ids=list(range(N_CORES)))

    layout = _cs_layout(_emission_order(_windows()[0]))
    rowsum = np.zeros(TWO_N, dtype=np.float64)
    pos_total = 0.0
    for r in range(N_CORES):
        rows = row_of[r]
        rs = res.results[r]["rowsums"].astype(np.float64)   # (128, 64)
        for k in range(N_SLOTS):
            srow = rs[:, k * 8:(k + 1) * 8].sum(axis=1)     # (128,)
            rowsum[rows[k * 128:(k + 1) * 128]] += srow
        cs = res.results[r]["colsums"].astype(np.float64)
        for (_, g), col in layout.items():
            rowsum[g * 128:(g + 1) * 128] += cs[:, col]
        pos_total += float(cs[:, POS_OFF:POS_OFF + 8].sum())

    sums = rowsum - float(np.exp(np.float32(T_INV), dtype=np.float32))
    loss = (np.sum(np.log(sums)) - T_INV * pos_total) / TWO_N
    return np.float32(loss)


# revision 25
# speedup vs baseline: 1.0933x; 1.0472x over previous
"""Contrastive volume loss (nn_ContrastiveVolumeLoss) on 8 Trainium2 cores.

Reference math:
  ind_k = floor(locations_k) @ [W, 1]
  G     = [emb_0.reshape(c,HW)[:, ind_0] | emb_1.reshape(c,HW)[:, ind_1]]
  sim   = G^T G                       (2n x 2n, G is channel-major (64, 8192))
  S_i   = sum_j exp(sim_ij / T) - e^(1/T)
  loss  = (sum_i log S_i - (2/T) sum_u sim[u, u+n]) / (2n)

Sharding: the host computes indices and slices the gathered point embeddings
(pure data staging). Each core owns 8 row-tiles of 128 rows (one per diagonal
work class, slot k's tile has diag region JD[k]); it computes the upper
trapezoid of the symmetric 8192x8192 sim matrix: slot k covers columns
[JD[k]*1024, 8192). Row sums of exp come from the activation accumulator or
DVE reduces; the lower triangle is recovered from column sums computed with
near-free tiny matmuls (exp-scratch as PE weights x ones, N=1) accumulated
in a pinned PSUM bank; the host merges rowsums + colsums + positive pairs.

Engine split (the scalar/Act engine is the PSUM-exp bottleneck; DVE is the
only other engine that can read PSUM):
  - Act: exact exp with fused row-sum accumulation on ~56% of columns.
  - DVE: Schraudolph approximate exp on the rest - one tensor_scalar per
    1024-col window computes int16 bits = sim*(T_inv*128/ln2) + 128*(127+s),
    which bitcast as bf16 is exp(sim/T) to ~1.8% rms (bias tuned via SIGMA).
  - Pool (gpsimd): bf16 add-trees combine 4 Schraudolph windows so DVE pays
    only one row-sum reduce per 4096 columns.
  - PE: bf16 sim matmuls + free colsum/pos tiny matmuls (cost model charges
    matmuls by output free size only).
"""

import numpy as np
import ml_dtypes

import concourse.bacc as bacc
import concourse.mybir as mybir
from concourse.tile import TileContext
from concourse.bass_utils import run_bass_kernel_spmd

N_CORES = 8
C = 64
HW = 256 * 256
N_PTS = 4096
TWO_N = 2 * N_PTS
T_INV = 10.0
W_IMG = 256

JD = [0, 7, 1, 6, 2, 5, 3, 4]      # diagonal region per slot
N_SLOTS = 8

SIGMA = -0.0579
A_CONST = float(T_INV * 128.0 / np.log(2.0))
B_CONST = float(128.0 * (127.0 + SIGMA))

_BF16 = ml_dtypes.bfloat16
_PROGRAM_CACHE = {}

# Per-slot window plan. 'A' windows -> Act exact exp (+accum rowsum);
# 'D' -> DVE Schraudolph window. Consecutive D windows of a slot form one
# group: Pool add-tree + a single DVE row-sum reduce. Sizes are columns
# relative to the slot start (global col JD[k]*1024). All windows <= 1536
# so every window streams through one double-buffered PSUM pool.
PLAN = {
    0: [("D", 1024), ("D", 1024), ("D", 1024), ("D", 1024),
        ("A", 1536), ("A", 1536), ("A", 1024)],
    1: [("A", 1024)],
    2: [("D", 1024), ("D", 1024), ("D", 1024), ("D", 1024),
        ("A", 1536), ("A", 1536)],
    3: [("A", 1024), ("A", 1024)],
    4: [("D", 1024), ("D", 1024), ("D", 1024), ("D", 1024),
        ("A", 1024), ("A", 1024)],
    5: [("A", 1536), ("A", 1536)],
    6: [("A", 1536), ("A", 1536), ("A", 1024), ("A", 1024)],
    7: [("D", 1024), ("D", 1024), ("D", 1024), ("D", 1024)],
}

POS_OFF = 0      # 8 pos columns, then one column per (window, block) pair


def _slot_tiles(r):
    """Global 128-row tile indices owned by core r, in slot order."""
    return [r, 63 - r, 8 + r, 55 - r, 16 + r, 47 - r, 24 + r, 39 - r]


def _windows():
    """Flat window list: (slot, glo, width, kind, acc_col, group_id).
    Consecutive D windows of a slot share a group (one rowsum reduce)."""
    out = []
    group_size = {}
    for k in range(N_SLOTS):
        base = JD[k] * 1024
        off = 0
        acc_i = 0
        gid = None
        for kind, w in PLAN[k]:
            if kind == "A":
                out.append((k, base + off, w, "A", k * 8 + acc_i, None))
                acc_i += 1
                gid = None
            else:
                if gid is None:
                    gid = (k, acc_i)
                    group_size[gid] = 0
                    acc_i += 1
                out.append((k, base + off, w, "D", gid[0] * 8 + gid[1], gid))
                group_size[gid] += 1
            off += w
        assert JD[k] * 1024 + off == TWO_N
    return out, group_size


def _emission_order(wins):
    """Interleave so the three PSUM pools pipeline: [D, A_big, D, A_small]."""
    d = [w for w in wins if w[3] == "D"]
    a_big = [w for w in wins if w[3] == "A" and w[2] > 1024]
    a_small = [w for w in wins if w[3] == "A" and w[2] <= 1024]
    order = []
    di = bi = si = 0
    while di < len(d) or bi < len(a_big) or si < len(a_small):
        if di < len(d):
            order.append(d[di]); di += 1
        if bi < len(a_big):
            order.append(a_big[bi]); bi += 1
        if di < len(d):
            order.append(d[di]); di += 1
        if si < len(a_small):
            order.append(a_small[si]); si += 1
    return order


def _cs_layout(order):
    """Assign each contributing (window, block) pair its own column of the
    pinned PSUM bank (no cross-instruction accumulation chains; the host
    sums the partials per global block). Returns {(wi, g): col}."""
    layout = {}
    col = 8  # cols 0..7 hold the pos dots
    for wi, (k, glo, w, kind, _, _) in enumerate(order):
        for b in range(w // 128):
            g = (glo // 128) + b
            if g // 8 == JD[k]:
                continue  # diagonal region: rowsum-only
            layout[(wi, g)] = col
            col += 1
    assert col <= 512
    return layout


def _build_program():
    nc = bacc.Bacc(
        "TRN2", target_bir_lowering=False, debug=False, num_devices=N_CORES
    )
    lhs_d = nc.dram_tensor("lhs", [C, 1024], mybir.dt.bfloat16,
                           kind="ExternalInput")
    rhs_d = nc.dram_tensor("rhs", [C, TWO_N], mybir.dt.bfloat16,
                           kind="ExternalInput")
    par_d = nc.dram_tensor("par", [C, 1024], mybir.dt.bfloat16,
                           kind="ExternalInput")
    ones_d = nc.dram_tensor("ones", [128, 1], mybir.dt.bfloat16,
                            kind="ExternalInput")
    wins, group_size = _windows()
    order = _emission_order(wins)
    layout = _cs_layout(order)
    n_cs_cols = 8 + len(layout)
    rs_d = nc.dram_tensor("rowsums", [128, 64], mybir.dt.float32,
                          kind="ExternalOutput")
    cs_d = nc.dram_tensor("colsums", [128, n_cs_cols], mybir.dt.float32,
                          kind="ExternalOutput")

    with TileContext(nc) as tc:
        with (
            tc.tile_pool(name="const", bufs=1) as cpool,
            tc.tile_pool(name="ascr", bufs=4) as apool,
            tc.tile_pool(name="dscr", bufs=8) as dpool,
            tc.tile_pool(name="addp", bufs=3) as addpool,
            tc.tile_pool(name="pin", bufs=1, space="PSUM") as pinpool,
            tc.tile_pool(name="pa", bufs=1, space="PSUM") as pa,
            tc.tile_pool(name="pb", bufs=1, space="PSUM") as pb,
            tc.tile_pool(name="pc", bufs=1, space="PSUM") as pc,
        ):
            # ---- constants / inputs ----
            lhs_t = cpool.tile([C, 1024], mybir.dt.bfloat16, tag="lhs")
            nc.scalar.dma_start(lhs_t[:], lhs_d[:])
            ones_t = cpool.tile([128, 1], mybir.dt.bfloat16, tag="ones")
            nc.scalar.dma_start(ones_t[:], ones_d[:])
            par_t = cpool.tile([C, 1024], mybir.dt.bfloat16, tag="par")
            nc.scalar.dma_start(par_t[:], par_d[:])
            rhs_t = cpool.tile([C, TWO_N], mybir.dt.bfloat16, tag="rhs")
            for j in range(4):
                nc.sync.dma_start(rhs_t[:, j * 2048:(j + 1) * 2048],
                                  rhs_d[:, j * 2048:(j + 1) * 2048])

            # exp act-table warm-up during the input DMA window
            warm_t = cpool.tile([1, 1], mybir.dt.float32, tag="warm")
            nc.gpsimd.memset(warm_t[:], 0.0)
            nc.scalar.activation(warm_t[:], warm_t[:],
                                 mybir.ActivationFunctionType.Exp, scale=1.0)

            # rowsum accumulator columns (<=4 per slot) and pinned colsum bank
            acc = cpool.tile([128, 64], mybir.dt.float32, tag="acc")
            nc.gpsimd.memset(acc[:], 0.0)
            cs_acc = pinpool.tile([128, 512], mybir.dt.float32, tag="cs")

            # PE p-state warm-up: keep the tensor engine continuously busy
            # from t~0 so real fills run at full clock once inputs land.
            wrow = cpool.tile([1, 512], mybir.dt.bfloat16, tag="wrow")
            nc.gpsimd.memset(wrow[:], 1.0)
            wps = pa.tile([128, 1536], mybir.dt.float32, tag="pa")
            for _ in range(7):
                nc.tensor.matmul(wps[:1, :512], wrow[:, :1], wrow[:],
                                 start=True, stop=True)

            # ---- positive pairs: prod = lhs .* par, tiny-matmul col dots ----
            prod_t = cpool.tile([C, 1024], mybir.dt.bfloat16, tag="prod")
            nc.gpsimd.tensor_tensor(prod_t[:], lhs_t[:], par_t[:],
                                    mybir.AluOpType.mult)
            for b in range(8):
                nc.tensor.matmul(cs_acc[:, POS_OFF + b:POS_OFF + b + 1],
                                 prod_t[:, b * 128:(b + 1) * 128],
                                 ones_t[:C, :], start=True, stop=True)

            # ---- streamed windows ----
            # Tiny colsum matmuls and group reduces are deferred by LAG
            # windows: PE runs in program order, so emitting them inline
            # would stall the next tile's fill behind the exp that the
            # scratch depends on.
            group_scr = {}   # gid -> list of (width, int16 scratch tile)
            deferred = []
            LAG = 3
            pbc = [pb, pc]
            pbc_i = 0
            for wi, (k, glo, w, kind, acol, gid) in enumerate(order):
                lhsT = lhs_t[:, k * 128:(k + 1) * 128]
                if kind == "A" and w > 1024:
                    ptile = pa.tile([128, 1536], mybir.dt.float32, tag="pa")
                else:
                    pool_ = pbc[pbc_i % 2]
                    tag_ = "pb" if pbc_i % 2 == 0 else "pc"
                    pbc_i += 1
                    ptile = pool_.tile([128, 1024], mybir.dt.float32,
                                       tag=tag_)
                nmm = (w + 511) // 512
                for h in range(nmm):
                    cw = min(512, w - h * 512)
                    nc.tensor.matmul(ptile[:, h * 512:h * 512 + cw], lhsT,
                                     rhs_t[:, glo + h * 512:glo + h * 512 + cw],
                                     start=True, stop=True)

                if kind == "A":
                    scr = apool.tile([128, w], mybir.dt.bfloat16,
                                     tag=f"as{w}")
                    nc.scalar.activation(
                        scr[:], ptile[:, :w],
                        mybir.ActivationFunctionType.Exp, scale=T_INV,
                        accum_out=acc[:, acol:acol + 1])
                    scr_b = scr
                else:
                    scr = dpool.tile([128, w], mybir.dt.int16, tag=f"ds{w}")
                    nc.vector.tensor_scalar(
                        scr[:], ptile[:, :w], A_CONST, B_CONST,
                        mybir.AluOpType.mult, mybir.AluOpType.add)
                    group_scr.setdefault(gid, []).append((w, scr))
                    scr_b = None

                # colsum tiny matmuls (free on PE): scratch block as weights
                sb = scr[:].bitcast(mybir.dt.bfloat16) if kind == "D" else scr_b[:]

                def _tiny(sb=sb, k=k, glo=glo, w=w, wi=wi):
                    for b in range(w // 128):
                        g = (glo // 128) + b
                        if g // 8 == JD[k]:
                            continue
                        col = layout[(wi, g)]
                        nc.tensor.matmul(
                            cs_acc[:, col:col + 1],
                            sb[:, b * 128:(b + 1) * 128], ones_t[:],
                            start=True, stop=True)
                deferred.append(_tiny)

                # close out a finished D group: Pool add tree (Pool is
                # otherwise idle), deferred DVE reduce of the tree total.
                if gid is not None and len(group_scr.get(gid, ())) == group_size[gid]:
                    mem = group_scr.pop(gid)
                    if len(mem) == 1:
                        red_in = mem[0][1][:].bitcast(mybir.dt.bfloat16)
                    elif len(mem) == 2:
                        gw = mem[0][0]
                        stot = addpool.tile([128, gw], mybir.dt.bfloat16,
                                            tag="stot")
                        nc.gpsimd.tensor_tensor(
                            stot[:], mem[0][1][:].bitcast(mybir.dt.bfloat16),
                            mem[1][1][:].bitcast(mybir.dt.bfloat16),
                            mybir.AluOpType.add)
                        red_in = stot[:]
                    else:
                        assert len(mem) == 4
                        s0, s1, s2, s3 = [t[:].bitcast(mybir.dt.bfloat16)
                                          for _, t in mem]
                        gw = mem[0][0]
                        s01 = addpool.tile([128, gw], mybir.dt.bfloat16,
                                           tag="s01")
                        s23 = addpool.tile([128, gw], mybir.dt.bfloat16,
                                           tag="s23")
                        stot = addpool.tile([128, gw], mybir.dt.bfloat16,
                                            tag="stot")
                        nc.gpsimd.tensor_tensor(s01[:], s0, s1,
                                                mybir.AluOpType.add)
                        nc.gpsimd.tensor_tensor(s23[:], s2, s3,
                                                mybir.AluOpType.add)
                        nc.gpsimd.tensor_tensor(stot[:], s01[:], s23[:],
                                                mybir.AluOpType.add)
                        red_in = stot[:]

                    def _red(red_in=red_in, acol=acol):
                        nc.vector.tensor_reduce(acc[:, acol:acol + 1],
                                                red_in,
                                                axis=mybir.AxisListType.X,
                                                op=mybir.AluOpType.add)
                    deferred.append(_red)

                while len(deferred) > LAG:
                    deferred.pop(0)()

            for fn in deferred:
                fn()

            # ---- outputs ----
            cs_sb = cpool.tile([128, n_cs_cols], mybir.dt.float32,
                               tag="cs_sb")
            nc.vector.tensor_copy(cs_sb[:], cs_acc[:, :n_cs_cols])
            nc.sync.dma_start(rs_d[:], acc[:])
            nc.sync.dma_start(cs_d[:], cs_sb[:])

    nc.compile()
    nc._cs_layout_host = layout
    return nc


def kernel(emb_0, emb_1, locations_0, locations_1):
    emb_0 = np.asarray(emb_0)
    emb_1 = np.asarray(emb_1)
    locations_0 = np.asarray(locations_0)
    locations_1 = np.asarray(locations_1)

    strides = np.array([W_IMG, 1], dtype=np.float32)
    ind0 = (np.floor(locations_0[0]) @ strides).astype(np.int32)
    ind1 = (np.floor(locations_1[0]) @ strides).astype(np.int32)

    g0 = emb_0.reshape(C, HW)[:, ind0]
    g1 = emb_1.reshape(C, HW)[:, ind1]
    G = np.concatenate([g0, g1], axis=1).astype(_BF16)   # (64, 8192)
    P = np.concatenate([g1, g0], axis=1).astype(_BF16)   # partner columns

    if "nc" not in _PROGRAM_CACHE:
        _PROGRAM_CACHE["nc"] = _build_program()
    nc = _PROGRAM_CACHE["nc"]

    ones = np.ones((128, 1), dtype=_BF16)
    in_maps = []
    row_of = np.empty((N_CORES, 1024), dtype=np.int64)
    for r in range(N_CORES):
        tiles = _slot_tiles(r)
        rows = np.concatenate(
            [np.arange(mt * 128, (mt + 1) * 128) for mt in tiles])
        row_of[r] = rows
        in_maps.append({
            "lhs": np.ascontiguousarray(G[:, rows]),
            "rhs": G,
            "par": np.ascontiguousarray(P[:, rows]),
            "ones": ones,
        })

    res = run_bass_kernel_spmd(nc, in_maps, core_ids=list(range(N_CORES)))

    layout = _cs_layout(_emission_order(_windows()[0]))
    rowsum = np.zeros(TWO_N, dtype=np.float64)
    pos_total = 0.0
    for r in range(N_CORES):
        rows = row_of[r]
        rs = res.results[r]["rowsums"].astype(np.float64)   # (128, 64)
        for k in range(N_SLOTS):
            srow = rs[:, k * 8:(k + 1) * 8].sum(axis=1)     # (128,)
            rowsum[rows[k * 128:(k + 1) * 128]] += srow
        cs = res.results[r]["colsums"].astype(np.float64)
        for (_, g), col in layout.items():
            rowsum[g * 128:(g + 1) * 128] += cs[:, col]
        pos_total += float(cs[:, POS_OFF:POS_OFF + 8].sum())

    sums = rowsum - float(np.exp(np.float32(T_INV), dtype=np.float32))
    loss = (np.sum(np.log(sums)) - T_INV * pos_total) / TWO_N
    return np.float32(loss)
